# revision 25
# baseline (speedup 1.0000x reference)
"""ColBERT MaxSim retrieval kernel for Trainium2 (8 NeuronCores).

scores[b, n] = sum_{q active} max_{t active} cos(q_hidden[b,q], d_hidden[n,t])

Strategy (docs sharded across 8 cores, 128 docs each):
  host: queries and documents are l2-normalized on the host (norms commute
        with the max/sum), active query tokens packed into one 128-slot
        tile. Only ACTIVE doc tokens ship (~50% of them): per core, docs
        are sorted by active-token count; the per-slot max count across
        the 8 cores forms a common segment structure, so one SPMD program
        serves all cores. Segments are bin-packed into 1024-column PSUM
        tiles. Documents ship as fp8e4 in the DoubleRow layout (K=128
        folded to 64 partitions x 2 sub-rows).
  device, per 1024-col bin:
    - DMA fp8 bin (64 partitions x 2KB contiguous)
    - PE: 2 DoubleRow fp8 matmuls -> sim [128 qslots, 1024 cols] PSUM
    - DVE: per equal-size segment group, max over tokens -> mxall[:, slots]
  final: PE matmul mxall.T @ onehot -> [slot, batch] scores, DMA out;
  host un-permutes slots back to doc order.
"""

import os
import sys
from contextlib import ExitStack

import numpy as np
import ml_dtypes

sys.path.insert(0, "/opt/trn_rl_repo")

# ---- problem constants (hardcoded per contest contract) ----
B, Lq, N, Ld, K = 8, 32, 1024, 256, 128
NCORES = 8
D = N // NCORES          # 128 docs per core
QS = 128                 # packed query slots
BIN = 1024               # sim columns per PSUM tile
EPS = 1e-8

DMA_SPLIT = int(os.environ.get("KRN_DMA_SPLIT", "1"))
BACKEND = os.environ.get("KRN_BACKEND", "hw")     # hw | sim

_CACHE = {}
LAST_EXEC_NS = None


def _plan_structure(d_mask):
    """Common cross-core packing plan from the doc masks.

    Per core, docs sort by active-token count; rank j's common segment
    size S[j] is the max over cores of the j-th smallest count (order
    statistics across cores nearly coincide, so padding is tiny). Ranks
    are then first-fit-decreasing packed into 1024-column PSUM bins and
    relabeled in bin order (so mxall columns fill monotonically).

    Returns (orders, rank_of_label, bins): orders[c] sorts core c's
    docs; rank_of_label[L] maps mxall column L back to a sort rank;
    bins is a list of [(label, size, col_offset), ...] per PSUM tile.
    """
    counts = (d_mask > 0).sum(1)                   # [N]
    orders = []
    S = np.zeros(D, dtype=np.int64)
    for c in range(NCORES):
        cc = counts[c * D:(c + 1) * D]
        o = np.argsort(cc, kind="stable")
        orders.append(o)
        S = np.maximum(S, cc[o])
    assert S[0] >= 1, "empty docs unsupported"
    # first-fit-decreasing over ranks
    packs, used = [], []
    for j in sorted(range(D), key=lambda j: -int(S[j])):
        for bi in range(len(packs)):
            if used[bi] + S[j] <= BIN:
                packs[bi].append(j)
                used[bi] += int(S[j])
                break
        else:
            packs.append([j])
            used.append(int(S[j]))
    # order bins lightest-first (fastest start for the DMA/PE/DVE chain)
    # and second-lightest last (shortest dependency tail before the final
    # scores matmul)
    order = sorted(range(len(packs)), key=lambda bi: used[bi])
    if len(order) > 2:
        order = [order[0]] + order[2:] + [order[1]]
    packs = [packs[bi] for bi in order]
    used = [used[bi] for bi in order]

    bins, rank_of_label = [], []
    for bi, bb in enumerate(packs):
        bb.sort(key=lambda j: -int(S[j]))   # equal sizes adjacent
        # merge reduce groups: pad slot sizes up into the bin's slack so
        # consecutive slots share one segment size (fewer, larger DVE
        # tensor_reduce instructions; padded columns are zeros -> max>=0,
        # same semantics as masked tokens)
        slack = BIN - used[bi]
        sizes = []
        for j in bb:
            s = int(S[j])
            if sizes and sizes[-1] > s and sizes[-1] - s <= slack:
                slack -= sizes[-1] - s
                s = sizes[-1]
            sizes.append(s)
        off, slots = 0, []
        for j, s in zip(bb, sizes):
            slots.append((len(rank_of_label), s, off))
            off += s
            rank_of_label.append(j)
        bins.append(slots)
    return orders, np.array(rank_of_label), bins


def _build_program(bins):
    import concourse.bacc as bacc
    import concourse.mybir as mybir
    import concourse.tile as tile

    f32 = mybir.dt.float32
    bf16 = mybir.dt.bfloat16
    fp8 = mybir.dt.float8e4
    AL = mybir.AluOpType
    NBINS = len(bins)

    nc = bacc.Bacc("TRN2", target_bir_lowering=False)

    dnd = nc.dram_tensor("dnd", [64, NBINS * 2 * BIN], fp8, kind="ExternalInput")
    qd = nc.dram_tensor("qd", [64, 2 * QS], fp8, kind="ExternalInput")
    ohd = nc.dram_tensor("ohd", [QS, B], bf16, kind="ExternalInput")
    sc = nc.dram_tensor("scores", [D, B], f32, kind="ExternalOutput")

    with ExitStack() as ctx:
        tc = ctx.enter_context(tile.TileContext(nc))
        const = ctx.enter_context(tc.tile_pool(name="const", bufs=1))
        dpool = ctx.enter_context(tc.tile_pool(name="dpool", bufs=4))
        pssim = ctx.enter_context(tc.tile_pool(name="pssim", bufs=3, space="PSUM"))

        # q weights first (gates the first ldweights); onehot only feeds
        # the final matmul, so it loads via the idle scalar queue.
        q8t = const.tile([64, 2 * QS], fp8)
        nc.sync.dma_start(q8t, qd[:, :])
        qw = q8t.rearrange("p (i m) -> p i m", i=2)
        ohw = const.tile([QS, B], bf16)
        mxall = const.tile([QS, D], bf16)

        for b, slots in enumerate(bins):
            if b == 1:
                # onehot only feeds the final matmul; issue it now so it
                # sits behind bin 0's chunks in the scalar queue
                nc.scalar.dma_start(ohw, ohd[:, :])
            dft = dpool.tile([64, 2 * BIN], fp8)
            ns = 2 * BIN // DMA_SPLIT
            for s in range(DMA_SPLIT):
                # alternate issue queues: each DMA costs ~650ns of
                # sequencing, so odd chunks go through the idle scalar
                # queue (gpsimd's SWDGE path measured slower)
                engs = (nc.sync, nc.scalar)
                eng = engs[(b * DMA_SPLIT + s) % len(engs)]
                eng.dma_start(
                    dft[:, s * ns:(s + 1) * ns],
                    dnd[:, b * 2 * BIN + s * ns:b * 2 * BIN + (s + 1) * ns],
                )
            dv = dft.rearrange("p (i n) -> p i n", i=2)

            used = slots[-1][1] + slots[-1][2]   # columns actually occupied
            sim = pssim.tile([128, BIN], f32)
            for p2 in range(2):
                c0, c1 = p2 * 512, min((p2 + 1) * 512, used)
                if c1 <= c0:
                    continue
                nc.tensor.matmul(
                    sim[:, c0:c1],
                    qw, dv[:, :, c0:c1],
                    start=True, stop=True,
                    perf_mode=mybir.MatmulPerfMode.DoubleRow,
                    skip_group_check=True,
                )
            # per-segment max over tokens, grouped by equal segment size
            i = 0
            while i < len(slots):
                j0, sj, off = slots[i]
                g = 1
                while i + g < len(slots) and slots[i + g][1] == sj:
                    g += 1
                nc.vector.tensor_reduce(
                    mxall[:, j0:j0 + g],
                    sim[:, off:off + g * sj].rearrange("p (d t) -> p d t", d=g),
                    axis=mybir.AxisListType.X, op=AL.max,
                )
                i += g

        # ---- scores: [slot, batch] = mxall.T @ onehot, split by halves
        # so the first half runs before the last bins finish ----
        scp = pssim.tile([128, B], f32, tag="misc", bufs=1)
        scsb = const.tile([D, B], f32)
        for half in range(2):
            nc.tensor.matmul(
                scp[half * 64:(half + 1) * 64, :],
                mxall[:, half * 64:(half + 1) * 64], ohw,
                start=True, stop=True,
                tile_position=(0, 64 * half), skip_group_check=True,
            )
            nc.vector.tensor_copy(
                scsb[half * 64:(half + 1) * 64, :],
                scp[half * 64:(half + 1) * 64, :],
            )
        nc.sync.dma_start(sc[:, :], scsb)

    nc.finalize()
    return nc


def _get_program(bins):
    key = (DMA_SPLIT, tuple((j, s, o) for bb in bins for (j, s, o) in bb),
           tuple(len(bb) for bb in bins))
    if key not in _CACHE:
        _CACHE[key] = _build_program(bins)
    return _CACHE[key]


def _host_prep(q_hidden, q_mask, d_hidden, d_mask):
    """Normalize, pack active tokens; return (in_maps, orders, bins)."""
    q_hidden = np.asarray(q_hidden, dtype=np.float32)
    q_mask = np.asarray(q_mask)
    d_hidden = np.asarray(d_hidden, dtype=np.float32)
    d_mask = np.asarray(d_mask)

    qn = q_hidden / np.maximum(
        np.sqrt((q_hidden * q_hidden).sum(-1, keepdims=True)), EPS)
    dn = d_hidden / np.maximum(
        np.sqrt((d_hidden * d_hidden).sum(-1, keepdims=True)), EPS)

    # pack active query tokens (ones padding; padded slots killed by onehot)
    qf = qn.reshape(B * Lq, K)
    act = np.nonzero(q_mask.reshape(-1) > 0)[0]
    assert len(act) <= QS, f"active q tokens {len(act)} > {QS} unsupported"
    qpack = np.ones((QS, K), np.float32)
    qpack[: len(act)] = qf[act]
    onehot = np.zeros((QS, B), np.float32)
    onehot[np.arange(len(act)), act // Lq] = 1.0
    oh16 = onehot.astype(ml_dtypes.bfloat16)
    q_in = np.ascontiguousarray(
        qpack.T.reshape(64, 2 * QS)).astype(ml_dtypes.float8_e4m3)

    orders, rank_of_label, bins = _plan_structure(d_mask)
    NBINS = len(bins)

    in_maps = []
    for c in range(NCORES):
        x = np.zeros((K, NBINS * BIN), np.float32)
        for b, slots in enumerate(bins):
            for (lab, sj, off) in slots:
                doc = c * D + int(orders[c][rank_of_label[lab]])
                tok = dn[doc][d_mask[doc] > 0]          # [count, K]
                x[:, b * BIN + off:b * BIN + off + len(tok)] = tok.T
        xf = x.reshape(64, 2, NBINS, BIN).transpose(0, 2, 1, 3)
        d_in = np.ascontiguousarray(
            xf.reshape(64, NBINS * 2 * BIN)).astype(ml_dtypes.float8_e4m3)
        in_maps.append({"dnd": d_in, "qd": q_in, "ohd": oh16})
    return in_maps, orders, rank_of_label, bins


def _run_sim(nc, in_maps):
    from concourse.bass_interp import CoreSim
    results = []
    for m in in_maps:
        sim = CoreSim(nc)
        for k, v in m.items():
            sim.tensor(k)[:] = v
        sim.simulate(check_with_hw=False)
        results.append({"scores": np.array(sim.tensor("scores"))})
    return results


def kernel(q_hidden, q_mask, d_hidden, d_mask):
    global LAST_EXEC_NS
    from concourse.bass_utils import run_bass_kernel_spmd

    in_maps, orders, rank_of_label, bins = _host_prep(
        q_hidden, q_mask, d_hidden, d_mask)
    nc = _get_program(bins)

    if BACKEND == "sim":
        results = _run_sim(nc, in_maps)
    else:
        kw = {}
        if os.environ.get("KRN_TMPDIR"):
            kw["tmpdir"] = os.environ["KRN_TMPDIR"]
        br = run_bass_kernel_spmd(nc, in_maps, core_ids=list(range(NCORES)), **kw)
        if br.exec_time_ns is not None:
            LAST_EXEC_NS = br.exec_time_ns
        results = br.results

    scores = np.empty((B, N), np.float32)
    for c in range(NCORES):
        out_c = results[c]["scores"]                   # [label, B]
        doc_of_label = orders[c][rank_of_label]        # label -> core doc
        scores[:, c * D:(c + 1) * D][:, doc_of_label] = out_c.T
    return scores


if __name__ == "__main__":
    # smoke build with a synthetic uniform structure
    bins = [[(j * 7 + i, 146, i * 146) for i in range(7)]
            for j in range(18)]
    bins = [bb for bb in bins if bb[0][0] < D]
    nc = _get_program([[t for t in bb if t[0] < D] for bb in bins])
    print("program built OK")


# revision 28
# speedup vs baseline: 1.0061x; 1.0061x over previous
"""ColBERT MaxSim retrieval kernel for Trainium2 (8 NeuronCores).

scores[b, n] = sum_{q active} max_{t active} cos(q_hidden[b,q], d_hidden[n,t])

Strategy (docs sharded across 8 cores, 128 docs each):
  host: queries and documents are l2-normalized on the host (norms commute
        with the max/sum), active query tokens packed into one 128-slot
        tile. Only ACTIVE doc tokens ship (~50% of them): per core, docs
        are sorted by active-token count; the per-slot max count across
        the 8 cores forms a common segment structure, so one SPMD program
        serves all cores. Segments are bin-packed into 1024-column PSUM
        tiles. Documents ship as fp8e4 in the DoubleRow layout (K=128
        folded to 64 partitions x 2 sub-rows).
  device, per 1024-col bin:
    - DMA fp8 bin (64 partitions x 2KB contiguous)
    - PE: 2 DoubleRow fp8 matmuls -> sim [128 qslots, 1024 cols] PSUM
    - DVE: per equal-size segment group, max over tokens -> mxall[:, slots]
  final: PE matmul mxall.T @ onehot -> [slot, batch] scores, DMA out;
  host un-permutes slots back to doc order.
"""

import os
import sys
from contextlib import ExitStack

import numpy as np
import ml_dtypes

sys.path.insert(0, "/opt/trn_rl_repo")

# ---- problem constants (hardcoded per contest contract) ----
B, Lq, N, Ld, K = 8, 32, 1024, 256, 128
NCORES = 8
D = N // NCORES          # 128 docs per core
QS = 128                 # packed query slots
BIN = 1024               # sim columns per PSUM tile
EPS = 1e-8

DMA_SPLIT = int(os.environ.get("KRN_DMA_SPLIT", "1"))
BACKEND = os.environ.get("KRN_BACKEND", "hw")     # hw | sim

_CACHE = {}
LAST_EXEC_NS = None


def _plan_structure(d_mask):
    """Common cross-core packing plan from the doc masks.

    Per core, docs sort by active-token count; rank j's common segment
    size S[j] is the max over cores of the j-th smallest count (order
    statistics across cores nearly coincide, so padding is tiny). Ranks
    are then first-fit-decreasing packed into 1024-column PSUM bins and
    relabeled in bin order (so mxall columns fill monotonically).

    Returns (orders, rank_of_label, bins): orders[c] sorts core c's
    docs; rank_of_label[L] maps mxall column L back to a sort rank;
    bins is a list of [(label, size, col_offset), ...] per PSUM tile.
    """
    counts = (d_mask > 0).sum(1)                   # [N]
    orders = []
    S = np.zeros(D, dtype=np.int64)
    for c in range(NCORES):
        cc = counts[c * D:(c + 1) * D]
        o = np.argsort(cc, kind="stable")
        orders.append(o)
        S = np.maximum(S, cc[o])
    assert S[0] >= 1, "empty docs unsupported"
    # first-fit-decreasing over ranks
    packs, used = [], []
    for j in sorted(range(D), key=lambda j: -int(S[j])):
        for bi in range(len(packs)):
            if used[bi] + S[j] <= BIN:
                packs[bi].append(j)
                used[bi] += int(S[j])
                break
        else:
            packs.append([j])
            used.append(int(S[j]))
    bins, rank_of_label = [], []
    for bi, bb in enumerate(packs):
        bb.sort(key=lambda j: -int(S[j]))   # equal sizes adjacent
        # merge reduce groups: pad slot sizes up into the bin's slack so
        # consecutive slots share one segment size (fewer, larger DVE
        # tensor_reduce instructions; padded columns are zeros -> max>=0,
        # same semantics as masked tokens)
        slack = BIN - used[bi]
        sizes = []
        for j in bb:
            s = int(S[j])
            if sizes and sizes[-1] > s and sizes[-1] - s <= slack:
                slack -= sizes[-1] - s
                s = sizes[-1]
            sizes.append(s)
        off, slots = 0, []
        for j, s in zip(bb, sizes):
            slots.append((len(rank_of_label), s, off))
            off += s
            rank_of_label.append(j)
        bins.append(slots)
    return orders, np.array(rank_of_label), bins


def _build_program(bins):
    import concourse.bacc as bacc
    import concourse.mybir as mybir
    import concourse.tile as tile

    f32 = mybir.dt.float32
    bf16 = mybir.dt.bfloat16
    fp8 = mybir.dt.float8e4
    AL = mybir.AluOpType
    NBINS = len(bins)

    nc = bacc.Bacc("TRN2", target_bir_lowering=False)

    dnd = nc.dram_tensor("dnd", [64, NBINS * 2 * BIN], fp8, kind="ExternalInput")
    qd = nc.dram_tensor("qd", [64, 2 * QS], fp8, kind="ExternalInput")
    ohd = nc.dram_tensor("ohd", [QS, B], bf16, kind="ExternalInput")
    sc = nc.dram_tensor("scores", [D, B], f32, kind="ExternalOutput")

    with ExitStack() as ctx:
        tc = ctx.enter_context(tile.TileContext(nc))
        const = ctx.enter_context(tc.tile_pool(name="const", bufs=1))
        dpool = ctx.enter_context(tc.tile_pool(name="dpool", bufs=4))
        pssim = ctx.enter_context(tc.tile_pool(name="pssim", bufs=3, space="PSUM"))
        psmisc = ctx.enter_context(tc.tile_pool(name="psmisc", bufs=1, space="PSUM"))

        # q weights first (gates the first ldweights); onehot only feeds
        # the final matmul, so it loads via the idle scalar queue.
        q8t = const.tile([64, 2 * QS], fp8)
        nc.sync.dma_start(q8t, qd[:, :])
        qw = q8t.rearrange("p (i m) -> p i m", i=2)
        ohw = const.tile([QS, B], bf16)
        mxall = const.tile([QS, D], bf16)

        for b, slots in enumerate(bins):
            if b == 1:
                # onehot only feeds the final matmul; issue it now so it
                # sits behind bin 0's chunks in the scalar queue
                nc.scalar.dma_start(ohw, ohd[:, :])
            dft = dpool.tile([64, 2 * BIN], fp8)
            ns = 2 * BIN // DMA_SPLIT
            for s in range(DMA_SPLIT):
                # alternate issue queues: each DMA costs ~650ns of
                # sequencing, so odd chunks go through the idle scalar
                # queue (gpsimd's SWDGE path measured slower)
                engs = (nc.sync, nc.scalar)
                eng = engs[(b * DMA_SPLIT + s) % len(engs)]
                eng.dma_start(
                    dft[:, s * ns:(s + 1) * ns],
                    dnd[:, b * 2 * BIN + s * ns:b * 2 * BIN + (s + 1) * ns],
                )
            dv = dft.rearrange("p (i n) -> p i n", i=2)

            used = slots[-1][1] + slots[-1][2]   # columns actually occupied
            sim = pssim.tile([128, BIN], f32)
            for p2 in range(2):
                c0, c1 = p2 * 512, min((p2 + 1) * 512, used)
                if c1 <= c0:
                    continue
                nc.tensor.matmul(
                    sim[:, c0:c1],
                    qw, dv[:, :, c0:c1],
                    start=True, stop=True,
                    perf_mode=mybir.MatmulPerfMode.DoubleRow,
                    skip_group_check=True,
                )
            # per-segment max over tokens, grouped by equal segment size
            i = 0
            while i < len(slots):
                j0, sj, off = slots[i]
                g = 1
                while i + g < len(slots) and slots[i + g][1] == sj:
                    g += 1
                nc.vector.tensor_reduce(
                    mxall[:, j0:j0 + g],
                    sim[:, off:off + g * sj].rearrange("p (d t) -> p d t", d=g),
                    axis=mybir.AxisListType.X, op=AL.max,
                )
                i += g

        # ---- scores: [slot, batch] = mxall.T @ onehot, split by halves
        # so the first half runs before the last bins finish ----
        scp = psmisc.tile([128, B], f32, tag="misc")
        scsb = const.tile([D, B], f32)
        for half in range(2):
            nc.tensor.matmul(
                scp[half * 64:(half + 1) * 64, :],
                mxall[:, half * 64:(half + 1) * 64], ohw,
                start=True, stop=True,
                tile_position=(0, 64 * half), skip_group_check=True,
            )
            nc.vector.tensor_copy(
                scsb[half * 64:(half + 1) * 64, :],
                scp[half * 64:(half + 1) * 64, :],
            )
        nc.sync.dma_start(sc[:, :], scsb)

    nc.finalize()
    return nc


def _get_program(bins):
    key = (DMA_SPLIT, tuple((j, s, o) for bb in bins for (j, s, o) in bb),
           tuple(len(bb) for bb in bins))
    if key not in _CACHE:
        _CACHE[key] = _build_program(bins)
    return _CACHE[key]


def _host_prep(q_hidden, q_mask, d_hidden, d_mask):
    """Normalize, pack active tokens; return (in_maps, orders, bins)."""
    q_hidden = np.asarray(q_hidden, dtype=np.float32)
    q_mask = np.asarray(q_mask)
    d_hidden = np.asarray(d_hidden, dtype=np.float32)
    d_mask = np.asarray(d_mask)

    qn = q_hidden / np.maximum(
        np.sqrt((q_hidden * q_hidden).sum(-1, keepdims=True)), EPS)
    dn = d_hidden / np.maximum(
        np.sqrt((d_hidden * d_hidden).sum(-1, keepdims=True)), EPS)

    # pack active query tokens (ones padding; padded slots killed by onehot)
    qf = qn.reshape(B * Lq, K)
    act = np.nonzero(q_mask.reshape(-1) > 0)[0]
    assert len(act) <= QS, f"active q tokens {len(act)} > {QS} unsupported"
    qpack = np.ones((QS, K), np.float32)
    qpack[: len(act)] = qf[act]
    onehot = np.zeros((QS, B), np.float32)
    onehot[np.arange(len(act)), act // Lq] = 1.0
    oh16 = onehot.astype(ml_dtypes.bfloat16)
    q_in = np.ascontiguousarray(
        qpack.T.reshape(64, 2 * QS)).astype(ml_dtypes.float8_e4m3)

    orders, rank_of_label, bins = _plan_structure(d_mask)
    NBINS = len(bins)

    in_maps = []
    for c in range(NCORES):
        x = np.zeros((K, NBINS * BIN), np.float32)
        for b, slots in enumerate(bins):
            for (lab, sj, off) in slots:
                doc = c * D + int(orders[c][rank_of_label[lab]])
                tok = dn[doc][d_mask[doc] > 0]          # [count, K]
                x[:, b * BIN + off:b * BIN + off + len(tok)] = tok.T
        xf = x.reshape(64, 2, NBINS, BIN).transpose(0, 2, 1, 3)
        d_in = np.ascontiguousarray(
            xf.reshape(64, NBINS * 2 * BIN)).astype(ml_dtypes.float8_e4m3)
        in_maps.append({"dnd": d_in, "qd": q_in, "ohd": oh16})
    return in_maps, orders, rank_of_label, bins


def _run_sim(nc, in_maps):
    from concourse.bass_interp import CoreSim
    results = []
    for m in in_maps:
        sim = CoreSim(nc)
        for k, v in m.items():
            sim.tensor(k)[:] = v
        sim.simulate(check_with_hw=False)
        results.append({"scores": np.array(sim.tensor("scores"))})
    return results


def kernel(q_hidden, q_mask, d_hidden, d_mask):
    global LAST_EXEC_NS
    from concourse.bass_utils import run_bass_kernel_spmd

    in_maps, orders, rank_of_label, bins = _host_prep(
        q_hidden, q_mask, d_hidden, d_mask)
    nc = _get_program(bins)

    if BACKEND == "sim":
        results = _run_sim(nc, in_maps)
    else:
        kw = {}
        if os.environ.get("KRN_TMPDIR"):
            kw["tmpdir"] = os.environ["KRN_TMPDIR"]
        br = run_bass_kernel_spmd(nc, in_maps, core_ids=list(range(NCORES)), **kw)
        if br.exec_time_ns is not None:
            LAST_EXEC_NS = br.exec_time_ns
        results = br.results

    scores = np.empty((B, N), np.float32)
    for c in range(NCORES):
        out_c = results[c]["scores"]                   # [label, B]
        doc_of_label = orders[c][rank_of_label]        # label -> core doc
        scores[:, c * D:(c + 1) * D][:, doc_of_label] = out_c.T
    return scores


if __name__ == "__main__":
    # smoke build with a synthetic uniform structure
    bins = [[(j * 7 + i, 146, i * 146) for i in range(7)]
            for j in range(18)]
    bins = [bb for bb in bins if bb[0][0] < D]
    nc = _get_program([[t for t in bb if t[0] < D] for bb in bins])
    print("program built OK")


# revision 29
# speedup vs baseline: 1.0623x; 1.0559x over previous
"""ColBERT MaxSim retrieval kernel for Trainium2 (8 NeuronCores).

scores[b, n] = sum_{q active} max_{t active} cos(q_hidden[b,q], d_hidden[n,t])

Strategy (docs sharded across 8 cores, 128 docs each):
  host: queries and documents are l2-normalized on the host (norms commute
        with the max/sum), active query tokens packed into one 128-slot
        tile. Only ACTIVE doc tokens ship (~50% of them): per core, docs
        are sorted by active-token count; the per-slot max count across
        the 8 cores forms a common segment structure, so one SPMD program
        serves all cores. Segments are bin-packed into 1024-column PSUM
        tiles. Documents ship as fp8e4 in the DoubleRow layout (K=128
        folded to 64 partitions x 2 sub-rows).
  device, per 1024-col bin:
    - DMA fp8 bin (64 partitions x 2KB contiguous)
    - PE: 2 DoubleRow fp8 matmuls -> sim [128 qslots, 1024 cols] PSUM
    - DVE: per equal-size segment group, max over tokens -> mxall[:, slots]
  final: PE matmul mxall.T @ onehot -> [slot, batch] scores, DMA out;
  host un-permutes slots back to doc order.
"""

import os
import sys
from contextlib import ExitStack

import numpy as np
import ml_dtypes

sys.path.insert(0, "/opt/trn_rl_repo")

# ---- problem constants (hardcoded per contest contract) ----
B, Lq, N, Ld, K = 8, 32, 1024, 256, 128
NCORES = 8
D = N // NCORES          # 128 docs per core
QS = 128                 # packed query slots
BIN = 1024               # sim columns per PSUM tile
EPS = 1e-8

DMA_SPLIT = int(os.environ.get("KRN_DMA_SPLIT", "1"))
BACKEND = os.environ.get("KRN_BACKEND", "hw")     # hw | sim

_CACHE = {}
LAST_EXEC_NS = None


def _plan_structure(d_mask):
    """Common cross-core packing plan from the doc masks.

    Per core, docs sort by active-token count; rank j's common segment
    size S[j] is the max over cores of the j-th smallest count (order
    statistics across cores nearly coincide, so padding is tiny). Ranks
    are then first-fit-decreasing packed into 1024-column PSUM bins and
    relabeled in bin order (so mxall columns fill monotonically).

    Returns (orders, rank_of_label, bins): orders[c] sorts core c's
    docs; rank_of_label[L] maps mxall column L back to a sort rank;
    bins is a list of [(label, size, col_offset), ...] per PSUM tile.
    """
    counts = (d_mask > 0).sum(1)                   # [N]
    orders = []
    S = np.zeros(D, dtype=np.int64)
    for c in range(NCORES):
        cc = counts[c * D:(c + 1) * D]
        o = np.argsort(cc, kind="stable")
        orders.append(o)
        S = np.maximum(S, cc[o])
    assert S[0] >= 1, "empty docs unsupported"
    # first-fit-decreasing over ranks
    packs, used = [], []
    for j in sorted(range(D), key=lambda j: -int(S[j])):
        for bi in range(len(packs)):
            if used[bi] + S[j] <= BIN:
                packs[bi].append(j)
                used[bi] += int(S[j])
                break
        else:
            packs.append([j])
            used.append(int(S[j]))
    bins, rank_of_label = [], []
    for bi, bb in enumerate(packs):
        bb.sort(key=lambda j: -int(S[j]))   # equal sizes adjacent
        # merge reduce groups: pad slot sizes up into the bin's slack so
        # consecutive slots share one segment size (fewer, larger DVE
        # tensor_reduce instructions; padded columns are zeros -> max>=0,
        # same semantics as masked tokens)
        slack = BIN - used[bi]
        sizes = []
        for j in bb:
            s = int(S[j])
            if sizes and sizes[-1] > s and sizes[-1] - s <= slack:
                slack -= sizes[-1] - s
                s = sizes[-1]
            sizes.append(s)
        off, slots = 0, []
        for j, s in zip(bb, sizes):
            slots.append((len(rank_of_label), s, off))
            off += s
            rank_of_label.append(j)
        bins.append(slots)
    return orders, np.array(rank_of_label), bins


def _build_program(bins):
    import concourse.bacc as bacc
    import concourse.mybir as mybir
    import concourse.tile as tile

    f32 = mybir.dt.float32
    bf16 = mybir.dt.bfloat16
    fp8 = mybir.dt.float8e4
    AL = mybir.AluOpType
    NBINS = len(bins)

    nc = bacc.Bacc("TRN2", target_bir_lowering=False)

    dnd = nc.dram_tensor("dnd", [64, NBINS * 2 * BIN], fp8, kind="ExternalInput")
    qd = nc.dram_tensor("qd", [64, 2 * QS], fp8, kind="ExternalInput")
    ohd = nc.dram_tensor("ohd", [QS, B], bf16, kind="ExternalInput")
    sc = nc.dram_tensor("scores", [D, B], f32, kind="ExternalOutput")

    with ExitStack() as ctx:
        tc = ctx.enter_context(tile.TileContext(nc))
        const = ctx.enter_context(tc.tile_pool(name="const", bufs=1))
        dpool = ctx.enter_context(tc.tile_pool(name="dpool", bufs=4))
        pssim = ctx.enter_context(tc.tile_pool(name="pssim", bufs=3, space="PSUM"))
        psmisc = ctx.enter_context(tc.tile_pool(name="psmisc", bufs=1, space="PSUM"))

        # q weights first (gates the first ldweights); onehot only feeds
        # the final matmul, so it loads via the idle scalar queue.
        q8t = const.tile([64, 2 * QS], fp8)
        nc.sync.dma_start(q8t, qd[:, :])
        qw = q8t.rearrange("p (i m) -> p i m", i=2)
        ohw = const.tile([QS, B], bf16)
        mxall = const.tile([QS, D], bf16)

        for b, slots in enumerate(bins):
            if b == 1:
                # onehot only feeds the final matmul; issue it now so it
                # sits behind bin 0's chunks in the scalar queue
                nc.scalar.dma_start(ohw, ohd[:, :])
            dft = dpool.tile([64, 2 * BIN], fp8)
            ns = 2 * BIN // DMA_SPLIT
            for s in range(DMA_SPLIT):
                # alternate issue queues: each DMA costs ~650ns of
                # sequencing, so odd chunks go through the idle scalar
                # queue (gpsimd's SWDGE path measured slower)
                engs = (nc.sync, nc.scalar)
                eng = engs[(b * DMA_SPLIT + s) % len(engs)]
                eng.dma_start(
                    dft[:, s * ns:(s + 1) * ns],
                    dnd[:, b * 2 * BIN + s * ns:b * 2 * BIN + (s + 1) * ns],
                )
            dv = dft.rearrange("p (i n) -> p i n", i=2)

            sim = pssim.tile([128, BIN], f32)
            for p2 in range(2):
                nc.tensor.matmul(
                    sim[:, p2 * 512:(p2 + 1) * 512],
                    qw, dv[:, :, p2 * 512:(p2 + 1) * 512],
                    start=True, stop=True,
                    perf_mode=mybir.MatmulPerfMode.DoubleRow,
                    skip_group_check=True,
                )
            # per-segment max over tokens, grouped by equal segment size
            i = 0
            while i < len(slots):
                j0, sj, off = slots[i]
                g = 1
                while i + g < len(slots) and slots[i + g][1] == sj:
                    g += 1
                nc.vector.tensor_reduce(
                    mxall[:, j0:j0 + g],
                    sim[:, off:off + g * sj].rearrange("p (d t) -> p d t", d=g),
                    axis=mybir.AxisListType.X, op=AL.max,
                )
                i += g

        # ---- scores: [slot, batch] = mxall.T @ onehot, split by halves
        # so the first half runs before the last bins finish ----
        scp = psmisc.tile([128, B], f32, tag="misc")
        scsb = const.tile([D, B], f32)
        for half in range(2):
            nc.tensor.matmul(
                scp[half * 64:(half + 1) * 64, :],
                mxall[:, half * 64:(half + 1) * 64], ohw,
                start=True, stop=True,
                tile_position=(0, 64 * half), skip_group_check=True,
            )
            nc.vector.tensor_copy(
                scsb[half * 64:(half + 1) * 64, :],
                scp[half * 64:(half + 1) * 64, :],
            )
        nc.sync.dma_start(sc[:, :], scsb)

    nc.finalize()
    return nc


def _get_program(bins):
    key = (DMA_SPLIT, tuple((j, s, o) for bb in bins for (j, s, o) in bb),
           tuple(len(bb) for bb in bins))
    if key not in _CACHE:
        _CACHE[key] = _build_program(bins)
    return _CACHE[key]


def _host_prep(q_hidden, q_mask, d_hidden, d_mask):
    """Normalize, pack active tokens; return (in_maps, orders, bins)."""
    q_hidden = np.asarray(q_hidden, dtype=np.float32)
    q_mask = np.asarray(q_mask)
    d_hidden = np.asarray(d_hidden, dtype=np.float32)
    d_mask = np.asarray(d_mask)

    qn = q_hidden / np.maximum(
        np.sqrt((q_hidden * q_hidden).sum(-1, keepdims=True)), EPS)
    dn = d_hidden / np.maximum(
        np.sqrt((d_hidden * d_hidden).sum(-1, keepdims=True)), EPS)

    # pack active query tokens (ones padding; padded slots killed by onehot)
    qf = qn.reshape(B * Lq, K)
    act = np.nonzero(q_mask.reshape(-1) > 0)[0]
    assert len(act) <= QS, f"active q tokens {len(act)} > {QS} unsupported"
    qpack = np.ones((QS, K), np.float32)
    qpack[: len(act)] = qf[act]
    onehot = np.zeros((QS, B), np.float32)
    onehot[np.arange(len(act)), act // Lq] = 1.0
    oh16 = onehot.astype(ml_dtypes.bfloat16)
    q_in = np.ascontiguousarray(
        qpack.T.reshape(64, 2 * QS)).astype(ml_dtypes.float8_e4m3)

    orders, rank_of_label, bins = _plan_structure(d_mask)
    NBINS = len(bins)

    in_maps = []
    for c in range(NCORES):
        x = np.zeros((K, NBINS * BIN), np.float32)
        for b, slots in enumerate(bins):
            for (lab, sj, off) in slots:
                doc = c * D + int(orders[c][rank_of_label[lab]])
                tok = dn[doc][d_mask[doc] > 0]          # [count, K]
                x[:, b * BIN + off:b * BIN + off + len(tok)] = tok.T
        xf = x.reshape(64, 2, NBINS, BIN).transpose(0, 2, 1, 3)
        d_in = np.ascontiguousarray(
            xf.reshape(64, NBINS * 2 * BIN)).astype(ml_dtypes.float8_e4m3)
        in_maps.append({"dnd": d_in, "qd": q_in, "ohd": oh16})
    return in_maps, orders, rank_of_label, bins


def _run_sim(nc, in_maps):
    from concourse.bass_interp import CoreSim
    results = []
    for m in in_maps:
        sim = CoreSim(nc)
        for k, v in m.items():
            sim.tensor(k)[:] = v
        sim.simulate(check_with_hw=False)
        results.append({"scores": np.array(sim.tensor("scores"))})
    return results


def kernel(q_hidden, q_mask, d_hidden, d_mask):
    global LAST_EXEC_NS
    from concourse.bass_utils import run_bass_kernel_spmd

    in_maps, orders, rank_of_label, bins = _host_prep(
        q_hidden, q_mask, d_hidden, d_mask)
    nc = _get_program(bins)

    if BACKEND == "sim":
        results = _run_sim(nc, in_maps)
    else:
        kw = {}
        if os.environ.get("KRN_TMPDIR"):
            kw["tmpdir"] = os.environ["KRN_TMPDIR"]
        br = run_bass_kernel_spmd(nc, in_maps, core_ids=list(range(NCORES)), **kw)
        if br.exec_time_ns is not None:
            LAST_EXEC_NS = br.exec_time_ns
        results = br.results

    scores = np.empty((B, N), np.float32)
    for c in range(NCORES):
        out_c = results[c]["scores"]                   # [label, B]
        doc_of_label = orders[c][rank_of_label]        # label -> core doc
        scores[:, c * D:(c + 1) * D][:, doc_of_label] = out_c.T
    return scores


if __name__ == "__main__":
    # smoke build with a synthetic uniform structure
    bins = [[(j * 7 + i, 146, i * 146) for i in range(7)]
            for j in range(18)]
    bins = [bb for bb in bins if bb[0][0] < D]
    nc = _get_program([[t for t in bb if t[0] < D] for bb in bins])
    print("program built OK")
